# revision 3
# baseline (speedup 1.0000x reference)
"""Causal self-attention (B=2, S=2048, E=1024, H=16, D=64) on 8 trn2 NeuronCores.

Sharding: tensor-parallel over heads — 2 heads per core. Each core computes
qkv^T = (W_qkv_c)^T x^T for its 3*128 qkv dims, runs causal attention for its
2 heads, and multiplies by its 128-row slice of W_proj, producing a partial
[4096, 1024] output. The host sums the 8 partials and adds b_proj.

Dataflow (everything transposed so no on-device transposes of x are needed):
  - host passes x^T [E, B*S]
  - qkv^T tiles [128, 3, 512] per 512-column chunk: matmul lhsT=W tile,
    rhs=x^T tile, DVE epilogue adds per-partition bias and rounds to fp32r
  - scores^T[k, q] = K @ Q^T via lhsT=K^T slice, rhs=Q^T slice (contraction=64)
  - causal mask added on the diagonal 128x512 blocks (DVE, additive -1e30)
  - exp via ACT (scale=1/8 folded in), no max-subtraction (scores ~N(0,1))
  - y^T and softmax denominator in one matmul: lhsT = [V | 1] (V transposed
    on the PE once per 128-tile), rhs = exp(scores^T); out [65, 512] PSUM
  - normalize: DVE reciprocal of row 64, gpsimd partition_broadcast, DVE mul
  - out = y^T.T @ W_proj via lhsT=y^T s-tile, rhs=W_proj slice

All matmul operands are fp32r (fp32 with 12 low mantissa bits rounded away,
~1.5e-4 relerr, full PE speed at N>=256). DMA into an fp32r tile rounds.
"""

import sys

if "/opt/trn_rl_repo" not in sys.path:
    sys.path.insert(0, "/opt/trn_rl_repo")

import numpy as np

import concourse.bass as bass  # noqa: F401  (bass types used via bacc)
import concourse.mybir as mybir
import concourse.tile as tile
from concourse import bacc
from concourse.bass_utils import run_bass_kernel_spmd
from concourse.masks import make_identity

B, S, E, H, D = 2, 2048, 1024, 16, 64
NCORES = 8
HPC = H // NCORES            # heads per core = 2
BS = B * S                   # 4096
CH = 512                     # column chunk of x^T / qkv^T / q-chunk
NCH = BS // CH               # 8 chunks
KT = S // 128                # 16 k-tiles per batch
f32 = mybir.dt.float32
f32r = mybir.dt.float32r
MASK_VAL = -1e30


def build_nc():
    nc = bacc.Bacc(None, target_bir_lowering=False)
    xT = nc.dram_tensor("xT", [E, BS], f32r, kind="ExternalInput")
    wqkv = nc.dram_tensor("wqkv", [E, 3 * 128], f32r, kind="ExternalInput")
    bqkv = nc.dram_tensor("bqkv", [128, 3], f32, kind="ExternalInput")
    wproj = nc.dram_tensor("wproj", [128, E], f32r, kind="ExternalInput")
    mask = nc.dram_tensor("mask", [128, 4, CH], f32, kind="ExternalInput")
    out = nc.dram_tensor("out", [BS, E], f32, kind="ExternalOutput")

    with tile.TileContext(nc) as tc:
        with (
            tc.tile_pool(name="singles", bufs=1) as singles,
            tc.tile_pool(name="xpool", bufs=16) as xpool,
            tc.tile_pool(name="ppool", bufs=3) as ppool,
            tc.tile_pool(name="npool", bufs=3) as npool,
            tc.tile_pool(name="opool", bufs=3) as opool,
            tc.tile_pool(name="ps_mm", bufs=2, space="PSUM") as ps_mm,
            tc.tile_pool(name="ps_s", bufs=2, space="PSUM") as ps_s,
            tc.tile_pool(name="ps_y", bufs=2, space="PSUM") as ps_y,
        ):
            # ---- persistent tiles ----
            wqkv_sb = singles.tile([128, 8, 384], f32r, tag="wqkv")
            nc.sync.dma_start(
                out=wqkv_sb, in_=wqkv.rearrange("(ko ki) m -> ki ko m", ki=128)
            )
            bqkv_sb = singles.tile([128, 3], f32, tag="bqkv")
            nc.sync.dma_start(out=bqkv_sb, in_=bqkv[:, :])
            wproj_sb = singles.tile([128, E], f32r, tag="wproj")
            nc.sync.dma_start(out=wproj_sb, in_=wproj[:, :])
            mask_sb = singles.tile([128, 4, CH], f32, tag="mask")
            nc.sync.dma_start(out=mask_sb, in_=mask[:, :, :])
            ident = singles.tile([128, 128], f32, tag="ident")
            make_identity(nc, ident[:])

            qkvT = [singles.tile([128, 3, CH], f32r, tag=f"qkvT{n}", name=f"qkvT{n}") for n in range(NCH)]
            # V_aug per batch: [128, kt, 130]; cols 0:64 head0 V, 64 ones,
            # 65:129 head1 V, 129 ones. Head h slice = [:, kt, 65h : 65h+65].
            vaug = [singles.tile([128, KT, 130], f32r, tag=f"vaug{b}", name=f"vaug{b}") for b in range(B)]
            ones_sb = singles.tile([128, KT], f32, tag="ones")
            nc.vector.memset(ones_sb[:], 1.0)
            for b in range(B):
                nc.vector.tensor_copy(out=vaug[b][:, :, 64:65], in_=ones_sb[:])
                nc.vector.tensor_copy(out=vaug[b][:, :, 129:130], in_=ones_sb[:])
            yT = [singles.tile([128, CH], f32r, tag=f"yT{n}", name=f"yT{n}") for n in range(NCH)]

            def qkv_chunk(n):
                xts = []
                for k in range(8):
                    t = xpool.tile([128, CH], f32r, tag="xt")
                    nc.sync.dma_start(
                        out=t, in_=xT[k * 128:(k + 1) * 128, n * CH:(n + 1) * CH]
                    )
                    xts.append(t)
                for m in range(3):
                    pm = ps_mm.tile([128, CH], f32, tag="mm")
                    for k in range(8):
                        nc.tensor.matmul(
                            pm[:],
                            wqkv_sb[:, k, m * 128:(m + 1) * 128],
                            xts[k][:],
                            start=(k == 0),
                            stop=(k == 7),
                        )
                    nc.vector.tensor_scalar_add(
                        out=qkvT[n][:, m, :], in0=pm[:], scalar1=bqkv_sb[:, m:m + 1]
                    )

            def v_transpose(b):
                for kt in range(KT):
                    n = b * 4 + kt // 4
                    off = (kt % 4) * 128
                    pt = ps_mm.tile([128, CH], f32, tag="mm")
                    nc.tensor.transpose(
                        pt[:, 0:128],
                        qkvT[n][:, 2, off:off + 128].bitcast(f32),
                        ident[:],
                    )
                    nc.vector.tensor_copy(out=vaug[b][:, kt, 0:64], in_=pt[:, 0:64])
                    nc.vector.tensor_copy(out=vaug[b][:, kt, 65:129], in_=pt[:, 64:128])

            def attention(b, h):
                hb = h * 64
                for qc in range(4):
                    nq = b * 4 + qc
                    ktmax = 4 * (qc + 1)
                    py = ps_y.tile([65, CH], f32, tag="y")
                    for kg in range(ktmax // 2):
                        pg = ps_s.tile([128, 2, CH], f32, tag="s")
                        for j in range(2):
                            kt = kg * 2 + j
                            nk = b * 4 + kt // 4
                            offk = (kt % 4) * 128
                            nc.tensor.matmul(
                                pg[:, j, :],
                                qkvT[nk][hb:hb + 64, 1, offk:offk + 128],
                                qkvT[nq][hb:hb + 64, 0, :],
                                start=True,
                                stop=True,
                            )
                        if kg >= 2 * qc:  # diagonal groups
                            j0 = (kg - 2 * qc) * 2
                            nc.vector.tensor_add(
                                out=pg[:, :, :],
                                in0=pg[:, :, :],
                                in1=mask_sb[:, j0:j0 + 2, :],
                            )
                        pt_sb = ppool.tile([128, 2, CH], f32r, tag="pT")
                        nc.scalar.activation(
                            out=pt_sb[:],
                            in_=pg[:, :, :],
                            func=mybir.ActivationFunctionType.Exp,
                            scale=0.125,
                        )
                        for j in range(2):
                            kt = kg * 2 + j
                            nc.tensor.matmul(
                                py[:],
                                vaug[b][:, kt, h * 65:h * 65 + 65],
                                pt_sb[:, j, :],
                                start=(kt == 0),
                                stop=(kt == ktmax - 1),
                                skip_group_check=True,
                            )
                    rec = npool.tile([1, CH], f32, tag="rec")
                    nc.vector.reciprocal(out=rec[:], in_=py[64:65, :])
                    bc = npool.tile([64, CH], f32, tag="bcast")
                    nc.gpsimd.partition_broadcast(out_ap=bc[:], in_ap=rec[:])
                    nc.vector.tensor_mul(
                        out=yT[nq][hb:hb + 64, :], in0=py[0:64, :], in1=bc[:]
                    )

            def proj(n):
                for st in range(4):
                    row0 = n * CH + st * 128
                    for j in range(2):
                        pp = ps_mm.tile([128, CH], f32, tag="mm")
                        nc.tensor.matmul(
                            pp[:],
                            yT[n][:, st * 128:(st + 1) * 128],
                            wproj_sb[:, j * CH:(j + 1) * CH],
                            start=True,
                            stop=True,
                        )
                        o_sb = opool.tile([128, CH], f32, tag="o")
                        nc.vector.tensor_copy(out=o_sb[:], in_=pp[:])
                        nc.sync.dma_start(
                            out=out[row0:row0 + 128, j * CH:(j + 1) * CH], in_=o_sb[:]
                        )

            # emission order = scheduling priority: pipeline batch 0's
            # attention against batch 1's qkv, proj per batch as soon as ready
            for n in range(4):
                qkv_chunk(n)
            v_transpose(0)
            for n in range(4, 8):
                qkv_chunk(n)
            for h in range(HPC):
                attention(0, h)
            for n in range(4):
                proj(n)
            v_transpose(1)
            for h in range(HPC):
                attention(1, h)
            for n in range(4, 8):
                proj(n)

    nc.finalize()
    return nc


def make_core_inputs(x, W_attn, b_attn, W_proj):
    """Host-side sharding: slice per-core weights, transpose x, build masks."""
    xT = np.ascontiguousarray(x.reshape(BS, E).T)  # [E, BS]

    # causal masks for the 4 diagonal 128-row blocks of a 512 q-chunk:
    # valid iff 128*o + i <= j
    i = np.arange(128)[:, None]
    j = np.arange(CH)[None, :]
    mask = np.stack(
        [np.where(128 * o + i <= j, 0.0, MASK_VAL) for o in range(4)], axis=1
    ).astype(np.float32)  # [128, 4, 512]

    in_maps = []
    for c in range(NCORES):
        cols = slice(128 * c, 128 * (c + 1))
        wqkv = np.ascontiguousarray(
            np.concatenate(
                [W_attn[:, cols], W_attn[:, E:][:, cols], W_attn[:, 2 * E:][:, cols]],
                axis=1,
            )
        )  # [E, 384]
        bq = np.stack(
            [b_attn[cols], b_attn[E:][cols], b_attn[2 * E:][cols]], axis=1
        ).astype(np.float32)  # [128, 3]
        wp = np.ascontiguousarray(W_proj[128 * c:128 * (c + 1), :])  # [128, E]
        in_maps.append(
            {"xT": xT, "wqkv": wqkv, "bqkv": bq, "wproj": wp, "mask": mask}
        )
    return in_maps


_NC_CACHE = None


def kernel_run(inputs, trace=False):
    """Run the bass kernel; returns (full_output, BassKernelResults)."""
    global _NC_CACHE
    x = np.asarray(inputs["x"], dtype=np.float32)
    W_attn = np.asarray(inputs["W_attn"], dtype=np.float32)
    b_attn = np.asarray(inputs["b_attn"], dtype=np.float32)
    W_proj = np.asarray(inputs["W_proj"], dtype=np.float32)
    b_proj = np.asarray(inputs["b_proj"], dtype=np.float32)

    if _NC_CACHE is None:
        _NC_CACHE = build_nc()
    nc = _NC_CACHE

    in_maps = make_core_inputs(x, W_attn, b_attn, W_proj)
    res = run_bass_kernel_spmd(
        nc, in_maps, core_ids=list(range(NCORES)), trace=trace
    )
    acc = np.zeros((BS, E), dtype=np.float64)
    for r in res.results:
        acc += r["out"]
    y = (acc + b_proj).astype(np.float32).reshape(B, S, E)
    return y, res


def kernel(**inputs):
    y, _ = kernel_run(inputs, trace=False)
    return y


if __name__ == "__main__":
    rng = np.random.default_rng(0)
    scale = 1.0 / np.sqrt(E)
    inputs = {
        "x": rng.standard_normal((B, S, E), dtype=np.float32),
        "W_attn": rng.standard_normal((E, 3 * E), dtype=np.float32) * scale,
        "b_attn": rng.standard_normal((3 * E,), dtype=np.float32) * 0.02,
        "W_proj": rng.standard_normal((E, E), dtype=np.float32) * scale,
        "b_proj": rng.standard_normal((E,), dtype=np.float32) * 0.02,
    }
    y = kernel(**inputs)
    print("kernel output", y.shape, y.dtype, float(np.abs(y).mean()))


# revision 8
# speedup vs baseline: 1.1016x; 1.1016x over previous
"""Causal self-attention (B=2, S=2048, E=1024, H=16, D=64) on 8 trn2 NeuronCores.

Sharding: tensor-parallel over heads — 2 heads per core. Each core computes
qkv^T = (W_qkv_c)^T x^T for its 3*128 qkv dims, runs causal attention for its
2 heads, and multiplies by its 128-row slice of W_proj, producing a partial
[4096, 1024] output. The host sums the 8 partials and adds b_proj.

Dataflow (everything transposed so no on-device transposes of x are needed):
  - host passes x^T [E, B*S]
  - qkv^T tiles [128, 3, 512] per 512-column chunk: matmul lhsT=W tile,
    rhs=x^T tile, DVE epilogue adds per-partition bias and rounds to fp32r
  - scores^T[k, q] = K @ Q^T via lhsT=K^T slice, rhs=Q^T slice (contraction=64)
  - causal mask added on the diagonal 128x512 blocks (DVE, additive -1e30)
  - exp via ACT (scale=1/8 folded in), no max-subtraction (scores ~N(0,1))
  - y^T and softmax denominator in one matmul: lhsT = [V | 1] (V transposed
    on the PE once per 128-tile), rhs = exp(scores^T); out [65, 512] PSUM
  - normalize: DVE reciprocal of row 64, gpsimd partition_broadcast, DVE mul
  - out = y^T.T @ W_proj via lhsT=y^T s-tile, rhs=W_proj slice

All matmul operands are fp32r (fp32 with 12 low mantissa bits rounded away,
~1.5e-4 relerr, full PE speed at N>=256). DMA into an fp32r tile rounds.
"""

import sys

if "/opt/trn_rl_repo" not in sys.path:
    sys.path.insert(0, "/opt/trn_rl_repo")

import numpy as np

import concourse.bass as bass  # noqa: F401  (bass types used via bacc)
import concourse.mybir as mybir
import concourse.tile as tile
from concourse import bacc
from concourse.bass_utils import run_bass_kernel_spmd
from concourse.masks import make_identity

B, S, E, H, D = 2, 2048, 1024, 16, 64
NCORES = 8
HPC = H // NCORES            # heads per core = 2
BS = B * S                   # 4096
CH = 512                     # column chunk of x^T / qkv^T / q-chunk
NCH = BS // CH               # 8 chunks
KT = S // 128                # 16 k-tiles per batch
f32 = mybir.dt.float32
f32r = mybir.dt.float32r
MASK_VAL = -1e30


def build_nc():
    nc = bacc.Bacc(None, target_bir_lowering=False)
    xT = nc.dram_tensor("xT", [E, BS], f32r, kind="ExternalInput")
    wqkv = nc.dram_tensor("wqkv", [E, 3 * 128], f32r, kind="ExternalInput")
    bqkv = nc.dram_tensor("bqkv", [128, 3], f32, kind="ExternalInput")
    wproj = nc.dram_tensor("wproj", [128, E], f32r, kind="ExternalInput")
    mask = nc.dram_tensor("mask", [128, 4, CH], f32, kind="ExternalInput")
    out = nc.dram_tensor("out", [BS, E], f32, kind="ExternalOutput")

    with tile.TileContext(nc) as tc:
        with (
            tc.tile_pool(name="singles", bufs=1) as singles,
            tc.tile_pool(name="xpool", bufs=16) as xpool,
            tc.tile_pool(name="ppool", bufs=3) as ppool,
            tc.tile_pool(name="npool", bufs=3) as npool,
            tc.tile_pool(name="opool", bufs=3) as opool,
            tc.tile_pool(name="ps_mm", bufs=2, space="PSUM") as ps_mm,
            tc.tile_pool(name="ps_s", bufs=2, space="PSUM") as ps_s,
            tc.tile_pool(name="ps_y", bufs=2, space="PSUM") as ps_y,
        ):
            # ---- persistent tiles ----
            wqkv_sb = singles.tile([128, 8, 384], f32r, tag="wqkv")
            nc.sync.dma_start(
                out=wqkv_sb, in_=wqkv.rearrange("(ko ki) m -> ki ko m", ki=128)
            )
            bqkv_sb = singles.tile([128, 3], f32, tag="bqkv")
            nc.sync.dma_start(out=bqkv_sb, in_=bqkv[:, :])
            wproj_sb = singles.tile([128, E], f32r, tag="wproj")
            nc.sync.dma_start(out=wproj_sb, in_=wproj[:, :])
            mask_sb = singles.tile([128, 4, CH], f32, tag="mask")
            nc.sync.dma_start(out=mask_sb, in_=mask[:, :, :])
            ident = singles.tile([128, 128], f32, tag="ident")
            make_identity(nc, ident[:])

            qkvT = [singles.tile([128, 3, CH], f32r, tag=f"qkvT{n}", name=f"qkvT{n}") for n in range(NCH)]
            # V_aug per batch: [128, kt, 130]; cols 0:64 head0 V, 64 ones,
            # 65:129 head1 V, 129 ones. Head h slice = [:, kt, 65h : 65h+65].
            vaug = [singles.tile([128, KT, 130], f32r, tag=f"vaug{b}", name=f"vaug{b}") for b in range(B)]
            ones_sb = singles.tile([128, KT], f32, tag="ones")
            nc.vector.memset(ones_sb[:], 1.0)
            for b in range(B):
                nc.vector.tensor_copy(out=vaug[b][:, :, 64:65], in_=ones_sb[:])
                nc.vector.tensor_copy(out=vaug[b][:, :, 129:130], in_=ones_sb[:])
            yT = [singles.tile([128, CH], f32r, tag=f"yT{n}", name=f"yT{n}") for n in range(NCH)]

            def qkv_chunk(n):
                xts = []
                for k in range(8):
                    t = xpool.tile([128, CH], f32r, tag="xt")
                    nc.sync.dma_start(
                        out=t, in_=xT[k * 128:(k + 1) * 128, n * CH:(n + 1) * CH]
                    )
                    xts.append(t)
                for m in range(3):
                    pm = ps_mm.tile([128, CH], f32, tag="mm")
                    for k in range(8):
                        nc.tensor.matmul(
                            pm[:],
                            wqkv_sb[:, k, m * 128:(m + 1) * 128],
                            xts[k][:],
                            start=(k == 0),
                            stop=(k == 7),
                        )
                    nc.vector.tensor_scalar_add(
                        out=qkvT[n][:, m, :], in0=pm[:], scalar1=bqkv_sb[:, m:m + 1]
                    )

            def v_transpose(b):
                for kt in range(KT):
                    n = b * 4 + kt // 4
                    off = (kt % 4) * 128
                    pt = ps_mm.tile([128, CH], f32, tag="mm")
                    nc.tensor.transpose(
                        pt[:, 0:128],
                        qkvT[n][:, 2, off:off + 128].bitcast(f32),
                        ident[:],
                    )
                    nc.scalar.copy(out=vaug[b][:, kt, 0:64], in_=pt[:, 0:64])
                    nc.scalar.copy(out=vaug[b][:, kt, 65:129], in_=pt[:, 64:128])

            def attention_qc(b, h, qc):
                    hb = h * 64
                    nq = b * 4 + qc
                    ktmax = 4 * (qc + 1)
                    py = ps_y.tile([65, CH], f32, tag="y")
                    for kg in range(ktmax // 2):
                        pg = ps_s.tile([128, 2, CH], f32, tag="s")
                        for j in range(2):
                            kt = kg * 2 + j
                            nk = b * 4 + kt // 4
                            offk = (kt % 4) * 128
                            nc.tensor.matmul(
                                pg[:, j, :],
                                qkvT[nk][hb:hb + 64, 1, offk:offk + 128],
                                qkvT[nq][hb:hb + 64, 0, :],
                                start=True,
                                stop=True,
                            )
                        if kg >= 2 * qc:  # diagonal groups
                            j0 = (kg - 2 * qc) * 2
                            nc.vector.tensor_add(
                                out=pg[:, :, :],
                                in0=pg[:, :, :],
                                in1=mask_sb[:, j0:j0 + 2, :],
                            )
                        pt_sb = ppool.tile([128, 2, CH], f32r, tag="pT")
                        nc.scalar.activation(
                            out=pt_sb[:],
                            in_=pg[:, :, :],
                            func=mybir.ActivationFunctionType.Exp,
                            scale=0.125,
                        )
                        for j in range(2):
                            kt = kg * 2 + j
                            nc.tensor.matmul(
                                py[:],
                                vaug[b][:, kt, h * 65:h * 65 + 65],
                                pt_sb[:, j, :],
                                start=(kt == 0),
                                stop=(kt == ktmax - 1),
                                skip_group_check=True,
                            )
                    den = npool.tile([1, CH], f32, tag="den")
                    nc.vector.tensor_copy(out=den[:], in_=py[64:65, :])
                    rec = npool.tile([1, CH], f32, tag="rec")
                    nc.vector.reciprocal_approx_fast(out=rec[:], in_=den[:])
                    bc = npool.tile([64, CH], f32, tag="bcast")
                    nc.gpsimd.partition_broadcast(out_ap=bc[:], in_ap=rec[:])
                    nc.vector.tensor_mul(
                        out=yT[nq][hb:hb + 64, :], in0=py[0:64, :], in1=bc[:]
                    )

            def proj(n):
                for st in range(4):
                    row0 = n * CH + st * 128
                    for j in range(2):
                        pp = ps_mm.tile([128, CH], f32, tag="mm")
                        nc.tensor.matmul(
                            pp[:],
                            yT[n][:, st * 128:(st + 1) * 128],
                            wproj_sb[:, j * CH:(j + 1) * CH],
                            start=True,
                            stop=True,
                        )
                        o_sb = opool.tile([128, CH], f32, tag="o")
                        nc.vector.tensor_copy(out=o_sb[:], in_=pp[:])
                        nc.sync.dma_start(
                            out=out[row0:row0 + 128, j * CH:(j + 1) * CH], in_=o_sb[:]
                        )

            # emission order = scheduling priority: pipeline batch 0's
            # attention against batch 1's qkv; proj interleaved per q-chunk
            # so the PE always has mask/exp-independent work (HAM warmth)
            for n in range(4):
                qkv_chunk(n)
            v_transpose(0)
            for n in range(4, 8):
                qkv_chunk(n)
            for qc in range(4):
                for h in range(HPC):
                    attention_qc(0, h, qc)
                proj(qc)
            v_transpose(1)
            for qc in range(4):
                for h in range(HPC):
                    attention_qc(1, h, qc)
                proj(4 + qc)

    nc.finalize()
    return nc


def make_core_inputs(x, W_attn, b_attn, W_proj):
    """Host-side sharding: slice per-core weights, transpose x, build masks."""
    xT = np.ascontiguousarray(x.reshape(BS, E).T)  # [E, BS]

    # causal masks for the 4 diagonal 128-row blocks of a 512 q-chunk:
    # valid iff 128*o + i <= j
    i = np.arange(128)[:, None]
    j = np.arange(CH)[None, :]
    mask = np.stack(
        [np.where(128 * o + i <= j, 0.0, MASK_VAL) for o in range(4)], axis=1
    ).astype(np.float32)  # [128, 4, 512]

    in_maps = []
    for c in range(NCORES):
        cols = slice(128 * c, 128 * (c + 1))
        wqkv = np.ascontiguousarray(
            np.concatenate(
                [W_attn[:, cols], W_attn[:, E:][:, cols], W_attn[:, 2 * E:][:, cols]],
                axis=1,
            )
        )  # [E, 384]
        bq = np.stack(
            [b_attn[cols], b_attn[E:][cols], b_attn[2 * E:][cols]], axis=1
        ).astype(np.float32)  # [128, 3]
        wp = np.ascontiguousarray(W_proj[128 * c:128 * (c + 1), :])  # [128, E]
        in_maps.append(
            {"xT": xT, "wqkv": wqkv, "bqkv": bq, "wproj": wp, "mask": mask}
        )
    return in_maps


_NC_CACHE = None


def kernel_run(inputs, trace=False):
    """Run the bass kernel; returns (full_output, BassKernelResults)."""
    global _NC_CACHE
    x = np.asarray(inputs["x"], dtype=np.float32)
    W_attn = np.asarray(inputs["W_attn"], dtype=np.float32)
    b_attn = np.asarray(inputs["b_attn"], dtype=np.float32)
    W_proj = np.asarray(inputs["W_proj"], dtype=np.float32)
    b_proj = np.asarray(inputs["b_proj"], dtype=np.float32)

    if _NC_CACHE is None:
        _NC_CACHE = build_nc()
    nc = _NC_CACHE

    in_maps = make_core_inputs(x, W_attn, b_attn, W_proj)
    res = run_bass_kernel_spmd(
        nc, in_maps, core_ids=list(range(NCORES)), trace=trace
    )
    acc = np.zeros((BS, E), dtype=np.float64)
    for r in res.results:
        acc += r["out"]
    y = (acc + b_proj).astype(np.float32).reshape(B, S, E)
    return y, res


def kernel(**inputs):
    y, _ = kernel_run(inputs, trace=False)
    return y


if __name__ == "__main__":
    rng = np.random.default_rng(0)
    scale = 1.0 / np.sqrt(E)
    inputs = {
        "x": rng.standard_normal((B, S, E), dtype=np.float32),
        "W_attn": rng.standard_normal((E, 3 * E), dtype=np.float32) * scale,
        "b_attn": rng.standard_normal((3 * E,), dtype=np.float32) * 0.02,
        "W_proj": rng.standard_normal((E, E), dtype=np.float32) * scale,
        "b_proj": rng.standard_normal((E,), dtype=np.float32) * 0.02,
    }
    y = kernel(**inputs)
    print("kernel output", y.shape, y.dtype, float(np.abs(y).mean()))


# revision 13
# speedup vs baseline: 1.2506x; 1.1353x over previous
"""Causal self-attention (B=2, S=2048, E=1024, H=16, D=64) on 8 trn2 NeuronCores.

Sharding: tensor-parallel over heads — 2 heads per core. Each core computes
qkv^T = (W_qkv_c)^T x^T for its 3*128 qkv dims, runs causal attention for its
2 heads, and multiplies by its 128-row slice of W_proj, producing a partial
[4096, 1024] output. The host sums the 8 partials and adds b_proj.

Dataflow (everything transposed so no on-device transposes of x are needed):
  - host passes x^T [E, B*S]
  - qkv^T tiles [128, 3, 512] per 512-column chunk: matmul lhsT=W tile,
    rhs=x^T tile, DVE epilogue adds per-partition bias and rounds to fp32r
  - scores^T[k, q] = K @ Q^T via lhsT=K^T slice, rhs=Q^T slice (contraction=64)
  - causal mask added on the diagonal 128x512 blocks (DVE, additive -1e30)
  - exp via ACT (scale=1/8 folded in), no max-subtraction (scores ~N(0,1))
  - y^T and softmax denominator in one matmul: lhsT = [V | 1] (V transposed
    on the PE once per 128-tile), rhs = exp(scores^T); out [65, 512] PSUM
  - normalize: DVE reciprocal of row 64, gpsimd partition_broadcast, DVE mul
  - out = y^T.T @ W_proj via lhsT=y^T s-tile, rhs=W_proj slice

All matmul operands are fp32r (fp32 with 12 low mantissa bits rounded away,
~1.5e-4 relerr, full PE speed at N>=256). DMA into an fp32r tile rounds.
"""

import sys

if "/opt/trn_rl_repo" not in sys.path:
    sys.path.insert(0, "/opt/trn_rl_repo")

import numpy as np

import concourse.bass as bass  # noqa: F401  (bass types used via bacc)
import concourse.mybir as mybir
import concourse.tile as tile
from concourse import bacc
from concourse.bass_utils import run_bass_kernel_spmd
from concourse.masks import make_identity

B, S, E, H, D = 2, 2048, 1024, 16, 64
NCORES = 8
HPC = H // NCORES            # heads per core = 2
BS = B * S                   # 4096
CH = 512                     # column chunk of x^T / qkv^T / q-chunk
NCH = BS // CH               # 8 chunks
KT = S // 128                # 16 k-tiles per batch
f32 = mybir.dt.float32
f32r = mybir.dt.float32r
bf16 = mybir.dt.bfloat16
DT = bf16 if __import__("os").environ.get("KDT", "bf16") == "bf16" else f32r
MASK_VAL = -1e30


def build_nc():
    nc = bacc.Bacc(None, target_bir_lowering=False)
    xT = nc.dram_tensor("xT", [E, BS], DT, kind="ExternalInput")
    wqkv = nc.dram_tensor("wqkv", [E, 3 * 128], DT, kind="ExternalInput")
    bqkv = nc.dram_tensor("bqkv", [128, 3], f32, kind="ExternalInput")
    wproj = nc.dram_tensor("wproj", [128, E], DT, kind="ExternalInput")
    mask = nc.dram_tensor("mask", [128, 4, CH], f32, kind="ExternalInput")
    out = nc.dram_tensor("out", [BS, E], f32, kind="ExternalOutput")

    with tile.TileContext(nc) as tc:
        with (
            tc.tile_pool(name="singles", bufs=1) as singles,
            tc.tile_pool(name="xpool", bufs=16) as xpool,
            tc.tile_pool(name="ppool", bufs=3) as ppool,
            tc.tile_pool(name="npool", bufs=3) as npool,
            tc.tile_pool(name="opool", bufs=3) as opool,
            tc.tile_pool(name="ps_mm", bufs=2, space="PSUM") as ps_mm,
            tc.tile_pool(name="ps_s", bufs=2, space="PSUM") as ps_s,
            tc.tile_pool(name="ps_y", bufs=2, space="PSUM") as ps_y,
        ):
            # ---- persistent tiles ----
            wqkv_sb = singles.tile([128, 8, 384], DT, tag="wqkv")
            nc.sync.dma_start(
                out=wqkv_sb, in_=wqkv.rearrange("(ko ki) m -> ki ko m", ki=128)
            )
            bqkv_sb = singles.tile([128, 3], f32, tag="bqkv")
            nc.sync.dma_start(out=bqkv_sb, in_=bqkv[:, :])
            wproj_sb = singles.tile([128, E], DT, tag="wproj")
            nc.sync.dma_start(out=wproj_sb, in_=wproj[:, :])
            mask_sb = singles.tile([128, 4, CH], f32, tag="mask")
            nc.sync.dma_start(out=mask_sb, in_=mask[:, :, :])
            ident = singles.tile([128, 128], DT, tag="ident")
            make_identity(nc, ident[:])

            qkvT = [singles.tile([128, 3, CH], DT, tag=f"qkvT{n}", name=f"qkvT{n}") for n in range(NCH)]
            # V_aug per batch: [128, kt, 130]; cols 0:64 head0 V, 64 ones,
            # 65:129 head1 V, 129 ones. Head h slice = [:, kt, 65h : 65h+65].
            vaug = [singles.tile([128, KT, 130], DT, tag=f"vaug{b}", name=f"vaug{b}") for b in range(B)]
            ones_sb = singles.tile([128, KT], f32, tag="ones")
            nc.vector.memset(ones_sb[:], 1.0)
            for b in range(B):
                nc.vector.tensor_copy(out=vaug[b][:, :, 64:65], in_=ones_sb[:])
                nc.vector.tensor_copy(out=vaug[b][:, :, 129:130], in_=ones_sb[:])
            yT = [singles.tile([128, CH], DT, tag=f"yT{n}", name=f"yT{n}") for n in range(NCH)]

            def qkv_chunk(n):
                xts = []
                for k in range(8):
                    t = xpool.tile([128, CH], DT, tag="xt")
                    nc.sync.dma_start(
                        out=t, in_=xT[k * 128:(k + 1) * 128, n * CH:(n + 1) * CH]
                    )
                    xts.append(t)
                for m in range(3):
                    pm = ps_mm.tile([128, CH], f32, tag="mm")
                    for k in range(8):
                        nc.tensor.matmul(
                            pm[:],
                            wqkv_sb[:, k, m * 128:(m + 1) * 128],
                            xts[k][:],
                            start=(k == 0),
                            stop=(k == 7),
                        )
                    nc.vector.tensor_scalar_add(
                        out=qkvT[n][:, m, :], in0=pm[:], scalar1=bqkv_sb[:, m:m + 1]
                    )

            def v_transpose(b):
                for kt in range(KT):
                    n = b * 4 + kt // 4
                    off = (kt % 4) * 128
                    pt = ps_mm.tile([128, CH], DT, tag="mm")
                    nc.tensor.transpose(
                        pt[:, 0:128],
                        qkvT[n][:, 2, off:off + 128],
                        ident[:],
                    )
                    nc.scalar.copy(out=vaug[b][:, kt, 0:64], in_=pt[:, 0:64])
                    nc.scalar.copy(out=vaug[b][:, kt, 65:129], in_=pt[:, 64:128])

            def attention_qc(b, h, qc):
                    hb = h * 64
                    nq = b * 4 + qc
                    ktmax = 4 * (qc + 1)
                    py = ps_y.tile([65, CH], f32, tag="y")
                    for kg in range(ktmax // 2):
                        pg = ps_s.tile([128, 2, CH], f32, tag="s")
                        for j in range(2):
                            kt = kg * 2 + j
                            nk = b * 4 + kt // 4
                            offk = (kt % 4) * 128
                            nc.tensor.matmul(
                                pg[:, j, :],
                                qkvT[nk][hb:hb + 64, 1, offk:offk + 128],
                                qkvT[nq][hb:hb + 64, 0, :],
                                start=True,
                                stop=True,
                            )
                        if kg >= 2 * qc:  # diagonal groups
                            j0 = (kg - 2 * qc) * 2
                            nc.vector.tensor_add(
                                out=pg[:, :, :],
                                in0=pg[:, :, :],
                                in1=mask_sb[:, j0:j0 + 2, :],
                            )
                        pt_sb = ppool.tile([128, 2, CH], DT, tag="pT")
                        nc.scalar.activation(
                            out=pt_sb[:],
                            in_=pg[:, :, :],
                            func=mybir.ActivationFunctionType.Exp,
                            scale=0.125,
                        )
                        for j in range(2):
                            kt = kg * 2 + j
                            nc.tensor.matmul(
                                py[:],
                                vaug[b][:, kt, h * 65:h * 65 + 65],
                                pt_sb[:, j, :],
                                start=(kt == 0),
                                stop=(kt == ktmax - 1),
                                skip_group_check=True,
                            )
                    den = npool.tile([1, CH], f32, tag="den")
                    nc.vector.tensor_copy(out=den[:], in_=py[64:65, :])
                    rec = npool.tile([1, CH], f32, tag="rec")
                    nc.vector.reciprocal_approx_fast(out=rec[:], in_=den[:])
                    bc = npool.tile([64, CH], f32, tag="bcast")
                    nc.gpsimd.partition_broadcast(out_ap=bc[:], in_ap=rec[:])
                    nc.vector.tensor_mul(
                        out=yT[nq][hb:hb + 64, :], in0=py[0:64, :], in1=bc[:]
                    )

            def proj(n):
                for st in range(4):
                    row0 = n * CH + st * 128
                    for j in range(2):
                        pp = ps_mm.tile([128, CH], f32, tag="mm")
                        nc.tensor.matmul(
                            pp[:],
                            yT[n][:, st * 128:(st + 1) * 128],
                            wproj_sb[:, j * CH:(j + 1) * CH],
                            start=True,
                            stop=True,
                        )
                        o_sb = opool.tile([128, CH], f32, tag="o")
                        nc.vector.tensor_copy(out=o_sb[:], in_=pp[:])
                        nc.sync.dma_start(
                            out=out[row0:row0 + 128, j * CH:(j + 1) * CH], in_=o_sb[:]
                        )

            # emission order = scheduling priority: pipeline batch 0's
            # attention against batch 1's qkv; proj interleaved per q-chunk
            # so the PE always has mask/exp-independent work (HAM warmth)
            for n in range(4):
                qkv_chunk(n)
            v_transpose(0)
            for n in range(4, 8):
                qkv_chunk(n)
            for qc in range(4):
                for h in range(HPC):
                    attention_qc(0, h, qc)
                proj(qc)
            v_transpose(1)
            for qc in range(4):
                for h in range(HPC):
                    attention_qc(1, h, qc)
                proj(4 + qc)

    nc.finalize()
    return nc


def make_core_inputs(x, W_attn, b_attn, W_proj):
    """Host-side sharding: slice per-core weights, transpose x, build masks."""
    np_dt = mybir.dt.np(DT)
    xT = np.ascontiguousarray(x.reshape(BS, E).T).astype(np_dt)  # [E, BS]

    # causal masks for the 4 diagonal 128-row blocks of a 512 q-chunk:
    # valid iff 128*o + i <= j
    i = np.arange(128)[:, None]
    j = np.arange(CH)[None, :]
    mask = np.stack(
        [np.where(128 * o + i <= j, 0.0, MASK_VAL) for o in range(4)], axis=1
    ).astype(np.float32)  # [128, 4, 512]

    in_maps = []
    for c in range(NCORES):
        cols = slice(128 * c, 128 * (c + 1))
        wqkv = np.ascontiguousarray(
            np.concatenate(
                [W_attn[:, cols], W_attn[:, E:][:, cols], W_attn[:, 2 * E:][:, cols]],
                axis=1,
            )
        ).astype(np_dt)  # [E, 384]
        bq = np.stack(
            [b_attn[cols], b_attn[E:][cols], b_attn[2 * E:][cols]], axis=1
        ).astype(np.float32)  # [128, 3]
        wp = np.ascontiguousarray(W_proj[128 * c:128 * (c + 1), :]).astype(np_dt)  # [128, E]
        in_maps.append(
            {"xT": xT, "wqkv": wqkv, "bqkv": bq, "wproj": wp, "mask": mask}
        )
    return in_maps


_NC_CACHE = None


def kernel_run(inputs, trace=False):
    """Run the bass kernel; returns (full_output, BassKernelResults)."""
    global _NC_CACHE
    x = np.asarray(inputs["x"], dtype=np.float32)
    W_attn = np.asarray(inputs["W_attn"], dtype=np.float32)
    b_attn = np.asarray(inputs["b_attn"], dtype=np.float32)
    W_proj = np.asarray(inputs["W_proj"], dtype=np.float32)
    b_proj = np.asarray(inputs["b_proj"], dtype=np.float32)

    if _NC_CACHE is None:
        _NC_CACHE = build_nc()
    nc = _NC_CACHE

    in_maps = make_core_inputs(x, W_attn, b_attn, W_proj)
    res = run_bass_kernel_spmd(
        nc, in_maps, core_ids=list(range(NCORES)), trace=trace
    )
    acc = np.zeros((BS, E), dtype=np.float64)
    for r in res.results:
        acc += r["out"]
    y = (acc + b_proj).astype(np.float32).reshape(B, S, E)
    return y, res


def kernel(**inputs):
    y, _ = kernel_run(inputs, trace=False)
    return y


if __name__ == "__main__":
    rng = np.random.default_rng(0)
    scale = 1.0 / np.sqrt(E)
    inputs = {
        "x": rng.standard_normal((B, S, E), dtype=np.float32),
        "W_attn": rng.standard_normal((E, 3 * E), dtype=np.float32) * scale,
        "b_attn": rng.standard_normal((3 * E,), dtype=np.float32) * 0.02,
        "W_proj": rng.standard_normal((E, E), dtype=np.float32) * scale,
        "b_proj": rng.standard_normal((E,), dtype=np.float32) * 0.02,
    }
    y = kernel(**inputs)
    print("kernel output", y.shape, y.dtype, float(np.abs(y).mean()))


# revision 15
# speedup vs baseline: 1.3183x; 1.0541x over previous
"""Causal self-attention (B=2, S=2048, E=1024, H=16, D=64) on 8 trn2 NeuronCores.

Sharding: tensor-parallel over heads — 2 heads per core. Each core computes
qkv^T = (W_qkv_c)^T x^T for its 3*128 qkv dims, runs causal attention for its
2 heads, and multiplies by its 128-row slice of W_proj, producing a partial
[4096, 1024] output. The host sums the 8 partials and adds b_proj.

Dataflow (everything transposed so no on-device transposes of x are needed):
  - host passes x^T [E, B*S]
  - qkv^T tiles [128, 3, 512] per 512-column chunk: matmul lhsT=W tile,
    rhs=x^T tile, DVE epilogue adds per-partition bias and rounds to fp32r
  - scores^T[k, q] = K @ Q^T via lhsT=K^T slice, rhs=Q^T slice (contraction=64)
  - causal mask added on the diagonal 128x512 blocks (DVE, additive -1e30)
  - exp via ACT (scale=1/8 folded in), no max-subtraction (scores ~N(0,1))
  - y^T and softmax denominator in one matmul: lhsT = [V | 1] (V transposed
    on the PE once per 128-tile), rhs = exp(scores^T); out [65, 512] PSUM
  - normalize: DVE reciprocal of row 64, gpsimd partition_broadcast, DVE mul
  - out = y^T.T @ W_proj via lhsT=y^T s-tile, rhs=W_proj slice

All matmul operands are fp32r (fp32 with 12 low mantissa bits rounded away,
~1.5e-4 relerr, full PE speed at N>=256). DMA into an fp32r tile rounds.
"""

import sys

if "/opt/trn_rl_repo" not in sys.path:
    sys.path.insert(0, "/opt/trn_rl_repo")

import numpy as np

import concourse.bass as bass  # noqa: F401  (bass types used via bacc)
import concourse.mybir as mybir
import concourse.tile as tile
from concourse import bacc
from concourse.bass_utils import run_bass_kernel_spmd
from concourse.masks import make_identity

B, S, E, H, D = 2, 2048, 1024, 16, 64
NCORES = 8
HPC = H // NCORES            # heads per core = 2
BS = B * S                   # 4096
CH = 512                     # column chunk of x^T / qkv^T / q-chunk
NCH = BS // CH               # 8 chunks
KT = S // 128                # 16 k-tiles per batch
f32 = mybir.dt.float32
f32r = mybir.dt.float32r
bf16 = mybir.dt.bfloat16
DT = bf16 if __import__("os").environ.get("KDT", "bf16") == "bf16" else f32r
MASK_VAL = -1e30


def build_nc():
    nc = bacc.Bacc(None, target_bir_lowering=False)
    xT = nc.dram_tensor("xT", [E, BS], DT, kind="ExternalInput")
    wqkv = nc.dram_tensor("wqkv", [E, 3 * 128], DT, kind="ExternalInput")
    bqkv = nc.dram_tensor("bqkv", [128, 3], f32, kind="ExternalInput")
    wproj = nc.dram_tensor("wproj", [128, E], DT, kind="ExternalInput")
    mask = nc.dram_tensor("mask", [128, 4, 2, CH], f32, kind="ExternalInput")
    out = nc.dram_tensor("out", [BS, E], f32, kind="ExternalOutput")

    with tile.TileContext(nc) as tc:
        with (
            tc.tile_pool(name="singles", bufs=1) as singles,
            tc.tile_pool(name="xpool", bufs=16) as xpool,
            tc.tile_pool(name="ppool", bufs=3) as ppool,
            tc.tile_pool(name="npool", bufs=3) as npool,
            tc.tile_pool(name="opool", bufs=3) as opool,
            tc.tile_pool(name="ps_mm", bufs=2, space="PSUM") as ps_mm,
            tc.tile_pool(name="ps_s", bufs=2, space="PSUM") as ps_s,
            tc.tile_pool(name="ps_y", bufs=2, space="PSUM") as ps_y,
        ):
            # ---- persistent tiles ----
            wqkv_sb = singles.tile([128, 8, 384], DT, tag="wqkv")
            nc.sync.dma_start(
                out=wqkv_sb, in_=wqkv.rearrange("(ko ki) m -> ki ko m", ki=128)
            )
            bqkv_sb = singles.tile([128, 3], f32, tag="bqkv")
            nc.sync.dma_start(out=bqkv_sb, in_=bqkv[:, :])
            wproj_sb = singles.tile([128, E], DT, tag="wproj")
            nc.sync.dma_start(out=wproj_sb, in_=wproj[:, :])
            mask_sb = singles.tile([128, 4, 2, CH], f32, tag="mask")
            nc.sync.dma_start(out=mask_sb, in_=mask[:, :, :, :])
            ident = singles.tile([128, 128], DT, tag="ident")
            make_identity(nc, ident[:])

            qkvT = [singles.tile([128, 3, CH], DT, tag=f"qkvT{n}", name=f"qkvT{n}") for n in range(NCH)]
            # V_aug per batch: [128, kt, 130]; cols 0:64 head0 V, 64 ones,
            # 65:129 head1 V, 129 ones. Head h slice = [:, kt, 65h : 65h+65].
            vaug = [singles.tile([128, KT, 130], DT, tag=f"vaug{b}", name=f"vaug{b}") for b in range(B)]
            ones_sb = singles.tile([128, KT], f32, tag="ones")
            nc.vector.memset(ones_sb[:], 1.0)
            for b in range(B):
                nc.vector.tensor_copy(out=vaug[b][:, :, 64:65], in_=ones_sb[:])
                nc.vector.tensor_copy(out=vaug[b][:, :, 129:130], in_=ones_sb[:])
            yT = [singles.tile([128, CH], DT, tag=f"yT{n}", name=f"yT{n}") for n in range(NCH)]

            def qkv_chunk(n):
                xts = []
                for k in range(8):
                    t = xpool.tile([128, CH], DT, tag="xt")
                    nc.sync.dma_start(
                        out=t, in_=xT[k * 128:(k + 1) * 128, n * CH:(n + 1) * CH]
                    )
                    xts.append(t)
                for m in range(3):
                    pm = ps_mm.tile([128, CH], f32, tag="mm")
                    for k in range(8):
                        nc.tensor.matmul(
                            pm[:],
                            wqkv_sb[:, k, m * 128:(m + 1) * 128],
                            xts[k][:],
                            start=(k == 0),
                            stop=(k == 7),
                        )
                    nc.vector.tensor_scalar_add(
                        out=qkvT[n][:, m, :], in0=pm[:], scalar1=bqkv_sb[:, m:m + 1]
                    )

            def v_transpose(b):
                for kt in range(KT):
                    n = b * 4 + kt // 4
                    off = (kt % 4) * 128
                    pt = ps_mm.tile([128, CH], DT, tag="mm")
                    nc.tensor.transpose(
                        pt[:, 0:128],
                        qkvT[n][:, 2, off:off + 128],
                        ident[:],
                    )
                    nc.scalar.copy(out=vaug[b][:, kt, 0:64], in_=pt[:, 0:64])
                    nc.scalar.copy(out=vaug[b][:, kt, 65:129], in_=pt[:, 64:128])

            def attention_qc(b, qc):
                # both heads together: packed K=64 score matmuls via PE
                # row-tiling (head h occupies array rows h*64..h*64+63)
                nq = b * 4 + qc
                ktmax = 4 * (qc + 1)
                py = [ps_y.tile([65, CH], f32, tag="y", name=f"py{b}_{qc}_{h}")
                      for h in range(2)]
                for kt in range(ktmax):
                    nk = b * 4 + kt // 4
                    offk = (kt % 4) * 128
                    pg = ps_s.tile([128, 2, CH], f32, tag="s")
                    for h in range(2):
                        hb = h * 64
                        nc.tensor.matmul(
                            pg[:, h, :],
                            qkvT[nk][hb:hb + 64, 1, offk:offk + 128],
                            qkvT[nq][hb:hb + 64, 0, :],
                            start=True,
                            stop=True,
                            tile_position=(hb, 0),
                        )
                    if kt >= 4 * qc:  # diagonal k-tiles
                        off = kt - 4 * qc
                        nc.vector.tensor_add(
                            out=pg[:, :, :],
                            in0=pg[:, :, :],
                            in1=mask_sb[:, off, :, :],
                        )
                    pt_sb = ppool.tile([128, 2, CH], DT, tag="pT")
                    nc.scalar.activation(
                        out=pt_sb[:],
                        in_=pg[:, :, :],
                        func=mybir.ActivationFunctionType.Exp,
                        scale=0.125,
                    )
                    for h in range(2):
                        nc.tensor.matmul(
                            py[h][:],
                            vaug[b][:, kt, h * 65:h * 65 + 65],
                            pt_sb[:, h, :],
                            start=(kt == 0),
                            stop=(kt == ktmax - 1),
                            skip_group_check=True,
                        )
                for h in range(2):
                    hb = h * 64
                    den = npool.tile([1, CH], f32, tag="den")
                    nc.vector.tensor_copy(out=den[:], in_=py[h][64:65, :])
                    rec = npool.tile([1, CH], f32, tag="rec")
                    nc.vector.reciprocal_approx_fast(out=rec[:], in_=den[:])
                    bc = npool.tile([64, CH], f32, tag="bcast")
                    nc.gpsimd.partition_broadcast(out_ap=bc[:], in_ap=rec[:])
                    nc.vector.tensor_mul(
                        out=yT[nq][hb:hb + 64, :], in0=py[h][0:64, :], in1=bc[:]
                    )

            def proj(n):
                for st in range(4):
                    row0 = n * CH + st * 128
                    for j in range(2):
                        pp = ps_mm.tile([128, CH], f32, tag="mm")
                        nc.tensor.matmul(
                            pp[:],
                            yT[n][:, st * 128:(st + 1) * 128],
                            wproj_sb[:, j * CH:(j + 1) * CH],
                            start=True,
                            stop=True,
                        )
                        o_sb = opool.tile([128, CH], f32, tag="o")
                        nc.vector.tensor_copy(out=o_sb[:], in_=pp[:])
                        nc.sync.dma_start(
                            out=out[row0:row0 + 128, j * CH:(j + 1) * CH], in_=o_sb[:]
                        )

            # emission order = scheduling priority: pipeline batch 0's
            # attention against batch 1's qkv; proj interleaved per q-chunk
            # so the PE always has mask/exp-independent work (HAM warmth)
            for n in range(4):
                qkv_chunk(n)
            v_transpose(0)
            for n in range(4, 8):
                qkv_chunk(n)
            for qc in range(4):
                attention_qc(0, qc)
                proj(qc)
            v_transpose(1)
            for qc in range(4):
                attention_qc(1, qc)
                proj(4 + qc)

    nc.finalize()
    return nc


def make_core_inputs(x, W_attn, b_attn, W_proj):
    """Host-side sharding: slice per-core weights, transpose x, build masks."""
    np_dt = mybir.dt.np(DT)
    xT = np.ascontiguousarray(x.reshape(BS, E).T).astype(np_dt)  # [E, BS]

    # causal masks for the 4 diagonal 128-row blocks of a 512 q-chunk:
    # valid iff 128*o + i <= j
    i = np.arange(128)[:, None]
    j = np.arange(CH)[None, :]
    mask = np.stack(
        [np.where(128 * o + i <= j, 0.0, MASK_VAL) for o in range(4)], axis=1
    ).astype(np.float32)  # [128, 4, 512]
    mask = np.repeat(mask[:, :, None, :], 2, axis=2)  # [128, 4, 2, 512] per head

    in_maps = []
    for c in range(NCORES):
        cols = slice(128 * c, 128 * (c + 1))
        wqkv = np.ascontiguousarray(
            np.concatenate(
                [W_attn[:, cols], W_attn[:, E:][:, cols], W_attn[:, 2 * E:][:, cols]],
                axis=1,
            )
        ).astype(np_dt)  # [E, 384]
        bq = np.stack(
            [b_attn[cols], b_attn[E:][cols], b_attn[2 * E:][cols]], axis=1
        ).astype(np.float32)  # [128, 3]
        wp = np.ascontiguousarray(W_proj[128 * c:128 * (c + 1), :]).astype(np_dt)  # [128, E]
        in_maps.append(
            {"xT": xT, "wqkv": wqkv, "bqkv": bq, "wproj": wp, "mask": mask}
        )
    return in_maps


_NC_CACHE = None


def kernel_run(inputs, trace=False):
    """Run the bass kernel; returns (full_output, BassKernelResults)."""
    global _NC_CACHE
    x = np.asarray(inputs["x"], dtype=np.float32)
    W_attn = np.asarray(inputs["W_attn"], dtype=np.float32)
    b_attn = np.asarray(inputs["b_attn"], dtype=np.float32)
    W_proj = np.asarray(inputs["W_proj"], dtype=np.float32)
    b_proj = np.asarray(inputs["b_proj"], dtype=np.float32)

    if _NC_CACHE is None:
        _NC_CACHE = build_nc()
    nc = _NC_CACHE

    in_maps = make_core_inputs(x, W_attn, b_attn, W_proj)
    res = run_bass_kernel_spmd(
        nc, in_maps, core_ids=list(range(NCORES)), trace=trace
    )
    acc = np.zeros((BS, E), dtype=np.float64)
    for r in res.results:
        acc += r["out"]
    y = (acc + b_proj).astype(np.float32).reshape(B, S, E)
    return y, res


def kernel(**inputs):
    y, _ = kernel_run(inputs, trace=False)
    return y


if __name__ == "__main__":
    rng = np.random.default_rng(0)
    scale = 1.0 / np.sqrt(E)
    inputs = {
        "x": rng.standard_normal((B, S, E), dtype=np.float32),
        "W_attn": rng.standard_normal((E, 3 * E), dtype=np.float32) * scale,
        "b_attn": rng.standard_normal((3 * E,), dtype=np.float32) * 0.02,
        "W_proj": rng.standard_normal((E, E), dtype=np.float32) * scale,
        "b_proj": rng.standard_normal((E,), dtype=np.float32) * 0.02,
    }
    y = kernel(**inputs)
    print("kernel output", y.shape, y.dtype, float(np.abs(y).mean()))
